# revision 11
# baseline (speedup 1.0000x reference)
"""Block-sparse attention (block-diagonal, BS=32) for trn2, 8 NeuronCores.

Sharding: data-parallel over batch B=32 -> 4 sequences per core.

Per-core pipeline (T=4096 tokens, C=768, H=12, d=64):
  xT   = transpose(x)                    (PE transpose, fp32 -> fp16)
  qkT  = qkv_w[:1536] @ xT + b           (fp16 matmul; channels on partitions)
  v    = x @ qkv_w[1536:].T + b          (fp16 matmul; tokens on partitions)
  per 128-token group, per head:
    sT   = k @ q.T                       (one [64,128]x[64,128] matmul)
    a    = exp(sT/8) * blockdiag_mask    (ACT exp + DVE mask multiply)
    o|Z  = a.T @ [v | 1]                 (one matmul; col 64 = softmax denom)
    o    = o * (1/Z)                     (per-partition scalar)
  oT   = transpose(o)                    (PE transpose)
  out  = oT.T @ proj_wT + b              (fp32r matmul)
"""

from contextlib import ExitStack

import numpy as np

import concourse.bass as bass
import concourse.mybir as mybir
import concourse.tile as tile
from concourse.bass_utils import run_bass_kernel_spmd
from concourse.masks import make_identity
from concourse.tile_scheduler import N_PROCS
from concourse.vector_clock import ScopedClock, VectorClock

F32 = mybir.dt.float32
F32R = mybir.dt.float32r
F16 = mybir.dt.float16
EXP = mybir.ActivationFunctionType.Exp
ADD = mybir.AluOpType.add
MULT = mybir.AluOpType.mult

B, N, C = 32, 1024, 768
H, D, BS = 12, 64, 32
NCORES = 8
BLOC = B // NCORES  # 4 sequences per core
T = BLOC * N        # 4096 tokens per core
P = 128
KC = C // P         # 6 contraction chunks
TT = 512            # stage-A token tile
NT = T // TT        # 8 token tiles per core
SCALE = 1.0 / 8.0   # 1/sqrt(64)
REPEAT = 1          # bench.py sets >1 to resolve HW time from wall clock


def _drain_and_barrier_split(self, tick_clock, wait_clock):
    # The tail SP Drain is a CTRL_NO_STRUCT instruction; this walrus build
    # rejects >2 sem waits on it. Absorb each proc's final tick on its own
    # SP nop (in-order engine) so the drain itself needs no waits.
    gc = tick_clock.global_clock
    for p in range(N_PROCS):
        if gc[p] <= 0:
            continue
        partial = VectorClock([gc[q] if q == p else 0 for q in range(N_PROCS)])
        nop = self.nc.sync.nop(nofuse=True, hint=f"drain_wait_p{p}")
        wait_clock.add_sem_waits(nop.ins, ScopedClock({None: partial}))
    self.nc.sync.drain()

    self.nc.all_engine_barrier()
    assert self.sems is not None
    popped = self.nc._tile_sem_poison_stack.pop()
    assert popped is self._sem_poison
    self.nc.clear_and_free_semaphores(list(self.sems.allocated().values()))
    self.nc.all_engine_barrier()


tile.TileContext._drain_and_barrier = _drain_and_barrier_split

_MAXW = 1  # this walrus build allows only 1 sem wait on S3_LW matmuls
_COMPUTE_ENGINES = {"PE", "Activation", "DVE", "Pool", "SP"}


def _split_waits_json(js):
    """Insert wait-absorbing NoOps before compute-engine instructions that
    carry more than _MAXW sem waits (engines execute their stream in order,
    so a preceding same-engine NoOp taking the excess waits is equivalent)."""
    import copy

    n_new = 0
    for fn in js["functions"]:
        for blk in fn["blocks"]:
            out = []
            for inst in blk["instructions"]:
                si = inst.get("sync_info")
                waits = si.get("on_wait", []) if si else []
                if len(waits) > _MAXW and inst["engine"] in _COMPUTE_ENGINES:
                    excess, keep = waits[:-_MAXW], waits[-_MAXW:]
                    while excess:
                        chunk, excess = excess[:_MAXW], excess[_MAXW:]
                        out.append(
                            {
                                "debug": inst.get("debug", 0),
                                "engine": inst["engine"],
                                "ins": [],
                                "name": f"{inst['name']}-wsplit{n_new}",
                                "opcode": "NoOp",
                                "outs": [],
                                "sync_info": {"on_update": [], "on_wait": chunk},
                            }
                        )
                        n_new += 1
                    si["on_wait"] = keep
                out.append(inst)
            blk["instructions"] = out
    return js


def _patch_json_bytes(nc):
    import json as _json

    orig = nc.to_json_bytes

    def patched():
        js = _json.loads(orig())
        _split_waits_json(js)
        return _json.dumps(js).encode()

    nc.to_json_bytes = patched


def build():
    nc = bass.Bass()
    x_d = nc.declare_dram_parameter("x", [T, C], F32, isOutput=False)
    qw_d = nc.declare_dram_parameter("qkv_w", [3 * C, C], F32, isOutput=False)
    qb_d = nc.declare_dram_parameter("qkv_b", [3 * C], F32, isOutput=False)
    pw_d = nc.declare_dram_parameter("proj_w", [C, C], F32, isOutput=False)
    pb_d = nc.declare_dram_parameter("proj_b", [C], F32, isOutput=False)
    out_d = nc.declare_dram_parameter("out", [T, C], F32, isOutput=True)

    with tile.TileContext(nc) as tc, ExitStack() as ctx:
        const = ctx.enter_context(tc.tile_pool(name="const", bufs=1))
        wtmp = ctx.enter_context(tc.tile_pool(name="wtmp", bufs=2))
        xin = ctx.enter_context(tc.tile_pool(name="xin", bufs=3))
        xtp = ctx.enter_context(tc.tile_pool(name="xt", bufs=2))
        qkp = ctx.enter_context(tc.tile_pool(name="qk", bufs=2))
        vtp = ctx.enter_context(tc.tile_pool(name="vt", bufs=2))
        otp = ctx.enter_context(tc.tile_pool(name="ot", bufs=1))
        oTp = ctx.enter_context(tc.tile_pool(name="oT", bufs=2))
        outp = ctx.enter_context(tc.tile_pool(name="outp", bufs=1))
        small = ctx.enter_context(tc.tile_pool(name="small", bufs=6))
        psA = ctx.enter_context(tc.tile_pool(name="psA", bufs=3, space="PSUM"))
        psSq = ctx.enter_context(tc.tile_pool(name="psSq", bufs=3, space="PSUM"))
        psAv = ctx.enter_context(tc.tile_pool(name="psAv", bufs=2, space="PSUM"))

        # ---- one-time constants ----
        ident = const.tile([P, P], F32)
        make_identity(nc, ident)

        mask = const.tile([P, P], F16)
        nc.vector.memset(mask, 0.0)
        for i in range(4):
            nc.vector.memset(mask[32 * i : 32 * i + 32, 32 * i : 32 * i + 32], 1.0)

        qkvb_pp = const.tile([P, 18], F32)  # qkv_b as [part, subtile]
        nc.sync.dma_start(qkvb_pp[:], qb_d.rearrange("(o p) -> p o", p=P))

        bv_bc = const.tile([P, C], F32)  # v-bias broadcast across partitions
        nc.sync.dma_start(bv_bc[:], qb_d[None, 2 * C : 3 * C].to_broadcast((P, C)))

        pb_bc = const.tile([P, C], F32)  # proj bias broadcast
        nc.sync.dma_start(pb_bc[:], pb_d[None, :].to_broadcast((P, C)))

        # ---- transposed weights ----
        qwT = const.tile([P, KC, 3 * C], F16)  # qkv_w.T  (fp16)
        for mc in range(18):
            wt = wtmp.tile([P, C], F32, tag="wload")
            nc.sync.dma_start(wt[:], qw_d[mc * P : (mc + 1) * P, :])
            for kc in range(KC):
                ps = psSq.tile([P, P], F32, tag="sq")
                nc.tensor.transpose(ps, wt[:, kc * P : (kc + 1) * P], ident)
                nc.any.tensor_copy(qwT[:, kc, mc * P : (mc + 1) * P], ps)

        pwT = const.tile([P, KC, C], F32R)  # proj_w.T (fp32, used as fp32r)
        for mc in range(KC):
            wt = wtmp.tile([P, C], F32, tag="wload")
            nc.sync.dma_start(wt[:], pw_d[mc * P : (mc + 1) * P, :])
            for kc in range(KC):
                ps = psSq.tile([P, P], F32, tag="sq")
                nc.tensor.transpose(ps, wt[:, kc * P : (kc + 1) * P], ident)
                nc.any.tensor_copy(pwT[:, kc, mc * P : (mc + 1) * P], ps)

        # ---- main loop over token tiles ----
        NSP = TT // P  # 128-token subtiles / groups per tile

        def one_pass():
          for tt in range(NT):
            t0 = tt * TT

            # A1: load x, transpose to xT (fp16)
            xt = xtp.tile([P, KC, TT], F16)
            for sp in range(NSP):
                xi = xin.tile([P, C], F32)
                nc.sync.dma_start(xi[:], x_d[t0 + sp * P : t0 + (sp + 1) * P, :])
                for kc in range(KC):
                    ps = psSq.tile([P, P], F32, tag="sq")
                    nc.tensor.transpose(ps, xi[:, kc * P : (kc + 1) * P], ident)
                    nc.any.tensor_copy(xt[:, kc, sp * P : (sp + 1) * P], ps)

            # A2: qkT = qkv_w[:1536] @ x.T + b   [chan-part, 12 sub, TT]
            qk = qkp.tile([P, 12, TT], F16)
            for m in range(12):
                ps = psA.tile([P, TT], F32, tag="A")
                for kc in range(KC):
                    nc.tensor.matmul(
                        ps,
                        lhsT=qwT[:, kc, m * P : (m + 1) * P],
                        rhs=xt[:, kc, :],
                        start=(kc == 0),
                        stop=(kc == KC - 1),
                    )
                nc.any.tensor_scalar(
                    qk[:, m, :], ps, qkvb_pp[:, m : m + 1], None, ADD
                )

            # A3: v (+bias, natural layout, ones col 64 per head)
            vt = vtp.tile([P, NSP, H, D + 1], F16)
            nc.vector.memset(vt[:, :, :, D : D + 1], 1.0)
            for sp in range(NSP):
                for half in range(2):
                    ps = psA.tile([P, TT], F32, tag="A")
                    nc0 = 2 * C + half * 384
                    for kc in range(KC):
                        nc.tensor.matmul(
                            ps[:, :384],
                            lhsT=xt[:, kc, sp * P : (sp + 1) * P],
                            rhs=qwT[:, kc, nc0 : nc0 + 384],
                            start=(kc == 0),
                            stop=(kc == KC - 1),
                        )
                    for hh in range(6):
                        h = half * 6 + hh
                        nc.any.tensor_tensor(
                            vt[:, sp, h, 0:D],
                            ps[:, hh * D : (hh + 1) * D],
                            bv_bc[:, h * D : (h + 1) * D],
                            ADD,
                        )

            # A4: block-diagonal attention per (group, head)
            ot = otp.tile([P, NSP, C], F32)
            for g in range(NSP):
                for h in range(H):
                    off = (h % 2) * D
                    mq = h // 2
                    mk = 6 + h // 2
                    ps1 = psSq.tile([P, P], F32, tag="sq")
                    nc.tensor.matmul(
                        ps1,
                        lhsT=qk[off : off + D, mk, g * P : (g + 1) * P],
                        rhs=qk[off : off + D, mq, g * P : (g + 1) * P],
                        start=True,
                        stop=True,
                    )
                    et = small.tile([P, P], F16, tag="exp")
                    nc.scalar.activation(et, ps1, EXP, bias=0.0, scale=SCALE)
                    at = small.tile([P, P], F16, tag="at")
                    nc.vector.tensor_tensor(at, et, mask, MULT)
                    ps2 = psAv.tile([P, D + 1], F32, tag="av")
                    nc.tensor.matmul(
                        ps2, lhsT=at, rhs=vt[:, g, h, :], start=True, stop=True
                    )
                    rz = small.tile([P, 1], F32, tag="rz")
                    nc.vector.reciprocal(rz, ps2[:, D : D + 1])
                    nc.any.tensor_scalar(
                        ot[:, g, h * D : (h + 1) * D], ps2[:, 0:D], rz, None, MULT
                    )

            # A5: oT = transpose(o)
            oT = oTp.tile([P, KC, TT], F32R)
            for g in range(NSP):
                for kc in range(KC):
                    ps = psSq.tile([P, P], F32, tag="sq")
                    nc.tensor.transpose(ps, ot[:, g, kc * P : (kc + 1) * P], ident)
                    nc.any.tensor_copy(oT[:, kc, g * P : (g + 1) * P], ps)

            # A6: out = o @ proj_w.T + b  (fp32r)
            outt = outp.tile([P, NSP, C], F32)
            for sp in range(NSP):
                for nh, nw in ((0, 512), (512, 256)):
                    ps = psA.tile([P, TT], F32, tag="A")
                    for kc in range(KC):
                        nc.tensor.matmul(
                            ps[:, :nw],
                            lhsT=oT[:, kc, sp * P : (sp + 1) * P],
                            rhs=pwT[:, kc, nh : nh + nw],
                            start=(kc == 0),
                            stop=(kc == KC - 1),
                        )
                    nc.any.tensor_tensor(
                        outt[:, sp, nh : nh + nw],
                        ps[:, :nw],
                        pb_bc[:, nh : nh + nw],
                        ADD,
                    )
                nc.sync.dma_start(
                    out_d[t0 + sp * P : t0 + (sp + 1) * P, :], outt[:, sp, :]
                )

        if REPEAT == 1:
            one_pass()
        else:
            with tc.For_i(0, REPEAT, 1):
                one_pass()

    return nc


_NC_CACHE = None


def kernel(x, qkv_w, qkv_b, proj_w, proj_b, _trace=False):
    global _NC_CACHE
    if _NC_CACHE is None:
        _NC_CACHE = build()
        _patch_json_bytes(_NC_CACHE)
    nc = _NC_CACHE

    x = np.ascontiguousarray(np.asarray(x, dtype=np.float32))
    qkv_w = np.ascontiguousarray(np.asarray(qkv_w, dtype=np.float32))
    qkv_b = np.ascontiguousarray(np.asarray(qkv_b, dtype=np.float32))
    proj_w = np.ascontiguousarray(np.asarray(proj_w, dtype=np.float32))
    proj_b = np.ascontiguousarray(np.asarray(proj_b, dtype=np.float32))

    in_maps = []
    for c in range(NCORES):
        xs = x[c * BLOC : (c + 1) * BLOC].reshape(T, C)
        in_maps.append(
            {
                "x": np.ascontiguousarray(xs),
                "qkv_w": qkv_w,
                "qkv_b": qkv_b,
                "proj_w": proj_w,
                "proj_b": proj_b,
            }
        )

    res = run_bass_kernel_spmd(nc, in_maps, list(range(NCORES)), trace=_trace)
    out = np.empty((B, N, C), dtype=np.float32)
    for c in range(NCORES):
        out[c * BLOC : (c + 1) * BLOC] = res.results[c]["out"].reshape(BLOC, N, C)
    if _trace:
        return out, res
    return out
